# revision 7
# baseline (speedup 1.0000x reference)
"""Trainium2 Bass kernel for nn_LocalAggregator (GNN message passing).

Reference computation (per batch b of 64; N=128 nodes, D=128 dim, A=1000 attrs):
  a_input = leaky_relu(h_i * h_j)                      # [N,N,D]
  e_k     = a_input @ a[:,k]                           # [N,N,4]
  alpha   = select e_{adj-1} where adj in 1..4 else -inf
  attn    = softmax(alpha, axis=-1)
  out     = attn @ h                                   # [N,D]
  attr    = A_attr_sess @ attr_embedding               # [N,D]

Key identities used on device:
  leaky_relu(x, 0.2) = 0.6*x + 0.4*|x|   and   x = h_i[d]*h_j[d]
  => e_k = Ht.T @ (0.6*a_k (.) Ht)  +  |Ht|.T @ (0.4*a_k (.) |Ht|)   (pure matmuls)
  e_k is symmetric in (i,j), so exp(e_k) is too. With host-side transposed
  adjacency masks, prodT[j,(k,i)] = 1[adj[i,j]==k+1] * exp(e_k[i,j]) is exactly
  the lhsT the output matmul needs; an appended ones-column in the rhs yields
  the softmax denominator in the same matmul.

All inputs are host-packed into the exact SBUF layout so every DMA is one
contiguous run per partition (large descriptors; DMA is bandwidth- not
descriptor-bound). Outputs land packed and are unpacked on host.

Sharding: data-parallel over batch, 8 batches per core on 8 NeuronCores.
"""

import os
import numpy as np
import ml_dtypes

import concourse.bass as bass
import concourse.bacc as bacc
import concourse.mybir as mybir
import concourse.tile as tile
from concourse.bass import ds
from concourse.bass_utils import run_bass_kernel_spmd

F32 = mybir.dt.float32
BF16 = mybir.dt.bfloat16
FP16 = mybir.dt.float16
F32R = mybir.dt.float32r
I8 = mybir.dt.int8
AF = mybir.ActivationFunctionType
OP = mybir.AluOpType

B, N, D, A = 64, 128, 128, 1000
NCORES = 8
B_LOC = B // NCORES          # 8 batches per core
GROUPS = 2                   # process batches in 2 groups of 4
GB = B_LOC // GROUPS         # 4 batches per group
NCHUNK = 8                   # attr contraction chunks
CHUNK = A // NCHUNK          # 125
DP = D + 4                   # hidden row padded: [0:D]=h, [D]=1.0, rest 0

_cache = {}


def _build():
    nc = bacc.Bacc("TRN2", target_bir_lowering=False, debug=False)

    # host-packed inputs (exact SBUF layouts)
    hid_d = nc.dram_tensor("hid", [N, B_LOC, DP], F32R, kind="ExternalInput")
    ast_d = nc.dram_tensor("ast", [N, B_LOC, 4 * N], I8, kind="ExternalInput")
    atr_d = nc.dram_tensor("atr", [CHUNK, B_LOC, NCHUNK, N], FP16, kind="ExternalInput")
    emb_d = nc.dram_tensor("emb", [CHUNK, NCHUNK, D], FP16, kind="ExternalInput")
    asc_d = nc.dram_tensor("asc", [D, 8], F32, kind="ExternalInput")
    idn_d = nc.dram_tensor("idn", [N, N], F32R, kind="ExternalInput")

    # packed outputs (host unpacks)
    out_d = nc.dram_tensor("out", [N, B_LOC, D], F32, kind="ExternalOutput")
    att_d = nc.dram_tensor("att", [D, B_LOC, N], F32, kind="ExternalOutput")

    with tile.TileContext(nc) as tc:
        with (
            tc.tile_pool(name="consts", bufs=1) as consts,
            tc.tile_pool(name="sbuf", bufs=2) as pool,
            tc.tile_pool(name="sbuf3", bufs=3) as pool3,
            tc.tile_pool(name="ps_t", bufs=2, space="PSUM") as ps_t,
            tc.tile_pool(name="ps_e", bufs=2, space="PSUM") as ps_e,
            tc.tile_pool(name="ps_o", bufs=2, space="PSUM") as ps_o,
            tc.tile_pool(name="ps_a", bufs=2, space="PSUM") as ps_a,
        ):
            ident = consts.tile([N, N], F32R)
            nc.sync.dma_start(out=ident[:], in_=idn_d[:])
            Hall = consts.tile([N, B_LOC, DP], F32R)  # [i, b, d | 1.0 | pad]
            nc.sync.dma_start(out=Hall[:], in_=hid_d[:])
            ASall = consts.tile([N, B_LOC, 4 * N], I8)  # [j, b, (k,i)] adj-shift
            nc.sync.dma_start(out=ASall[:], in_=ast_d[:])
            asc = consts.tile([D, 8], F32)  # [:,0:4]=0.6*a, [:,4:8]=0.4*a
            nc.sync.dma_start(out=asc[:], in_=asc_d[:])
            ATall = consts.tile([CHUNK, B_LOC, NCHUNK, N], FP16)  # [a, b, c, n]
            nc.scalar.dma_start(out=ATall[:], in_=atr_d[:])
            emb = consts.tile([CHUNK, NCHUNK, D], FP16)
            nc.scalar.dma_start(out=emb[:], in_=emb_d[:])

            for g in range(GROUPS):
                b0 = g * GB
                H4 = Hall[:, b0 : b0 + GB]
                AS4 = ASall[:, b0 : b0 + GB]
                AT4 = ATall[:, b0 : b0 + GB]

                # ---- transpose H for all 4 batches into one PSUM bank ----
                psT = ps_t.tile([N, GB * N], F32R)
                for b in range(GB):
                    nc.tensor.transpose(psT[:, ds(b * N, N)], H4[:, b, 0:D], ident[:])
                HT4 = pool.tile([D, GB * N], F32R)  # [d, (b,i)]
                nc.scalar.copy(HT4[:], psT[:])
                AB4 = pool.tile([D, GB * N], F32R)  # |HT|
                nc.scalar.activation(AB4[:], HT4[:], AF.Abs)

                # ---- U/V: per-partition scaled copies, stacked over k ----
                U = pool.tile([D, 4, GB * N], F32R)
                V = pool.tile([D, 4, GB * N], F32R)
                for k in range(4):
                    nc.vector.tensor_scalar_mul(U[:, k], HT4[:], asc[:, k : k + 1])
                    nc.vector.tensor_scalar_mul(V[:, k], AB4[:], asc[:, 4 + k : 5 + k])

                # ---- per-batch attention ----
                outS = pool.tile([N, GB, D], F32)
                for b in range(GB):
                    e4 = ps_e.tile([N, 4 * N], F32)  # e_k[p, f] stacked over k
                    nc.tensor.matmul(
                        e4[:].rearrange("p (k f) -> p k f", k=4),
                        lhsT=HT4[:, ds(b * N, N)],
                        rhs=U[:, :, ds(b * N, N)],
                        start=True,
                        stop=False,
                    )
                    nc.tensor.matmul(
                        e4[:].rearrange("p (k f) -> p k f", k=4),
                        lhsT=AB4[:, ds(b * N, N)],
                        rhs=V[:, :, ds(b * N, N)],
                        start=False,
                        stop=True,
                    )
                    exp4 = pool3.tile([N, 4 * N], F32)
                    nc.scalar.activation(exp4[:], e4[:], AF.Exp)

                    # prodT[j,(k,i)] = (ASt==0) * exp(e_k)   (uses e_k symmetry)
                    prodT = pool3.tile([N, 4 * N], F32R)
                    nc.vector.scalar_tensor_tensor(
                        out=prodT[:],
                        in0=AS4[:, b],
                        scalar=0.0,
                        in1=exp4[:],
                        op0=OP.is_equal,
                        op1=OP.mult,
                    )

                    # out[i, 0:D] = sum_kj prodT * h ; out[i, D] = rowsum
                    psO = ps_o.tile([N, DP], F32)
                    for k in range(4):
                        nc.tensor.matmul(
                            psO[:],
                            lhsT=prodT[:, ds(k * N, N)],
                            rhs=H4[:, b, :],
                            start=(k == 0),
                            stop=(k == 3),
                        )
                    rs = pool3.tile([N, 1], F32)
                    nc.vector.reciprocal(rs[:], psO[:, D : D + 1])
                    nc.scalar.activation(
                        outS[:, b], psO[:, 0:D], AF.Copy, bias=0.0, scale=rs[:]
                    )
                nc.scalar.dma_start(out=out_d[:, b0 : b0 + GB], in_=outS[:])

                # ---- attr matmuls: all 4 batches per chunk, one N=512 MM ----
                psA = ps_a.tile([D, GB, N], F32)
                for c in range(NCHUNK):
                    nc.tensor.matmul(
                        psA[:],
                        lhsT=emb[:, c],
                        rhs=AT4[:, :, c, :],
                        start=(c == 0),
                        stop=(c == NCHUNK - 1),
                    )
                atS = pool.tile([D, GB, N], F32)
                nc.scalar.copy(atS[:], psA[:])
                nc.scalar.dma_start(out=att_d[:, b0 : b0 + GB], in_=atS[:])

    nc.compile()
    return nc


def kernel(hidden, adj, a, A_attr_sess, attr_embedding):
    hidden = np.asarray(hidden, dtype=np.float32)
    adj = np.asarray(adj)
    a = np.asarray(a, dtype=np.float32)
    A_attr_sess = np.asarray(A_attr_sess, dtype=np.float32)
    attr_embedding = np.asarray(attr_embedding, dtype=np.float32)

    # ---- host-side packing (sharding-layer data movement) ----
    asc = np.concatenate([0.6 * a, 0.4 * a], axis=1).astype(np.float32)  # [D, 8]
    asc = np.ascontiguousarray(asc)

    # hid_p[core][i, b, :] = [h(b,i,:) | 1.0 | 0 0 0]
    hid_p = np.zeros((B, N, DP), np.float32)
    hid_p[:, :, 0:D] = hidden
    hid_p[:, :, D] = 1.0
    hid_p = np.ascontiguousarray(
        hid_p.reshape(NCORES, B_LOC, N, DP).transpose(0, 2, 1, 3)
    )  # [core, i, b_loc, DP]

    # ast_p[core][j, b, k, i] = adj[b][i, j] - (k+1)  (transposed adjacency)
    adjT = adj.astype(np.int32).transpose(0, 2, 1)  # [B, j, i]
    ast = (
        adjT[:, :, None, :]
        - np.array([1, 2, 3, 4], np.int32)[None, None, :, None]
    ).astype(np.int8)  # [B, j, 4, i]
    ast_p = np.ascontiguousarray(
        ast.reshape(NCORES, B_LOC, N, 4 * N).transpose(0, 2, 1, 3)
    )  # [core, j, b_loc, 4*N]

    # atr_p[core][p, b, c, n] = A_attr_sess[b, n, c*CHUNK+p]
    atr = (
        A_attr_sess.astype(np.float16)
        .transpose(2, 0, 1)
        .reshape(NCHUNK, CHUNK, B, N)
    )  # [c, p, B, n]
    atr_p = np.ascontiguousarray(
        atr.transpose(2, 1, 0, 3)  # [B, p, c, n]
        .reshape(NCORES, B_LOC, CHUNK, NCHUNK, N)
        .transpose(0, 2, 1, 3, 4)
    )  # [core, p, b_loc, c, n]

    emb_p = np.ascontiguousarray(
        attr_embedding.astype(np.float16).reshape(NCHUNK, CHUNK, D).transpose(1, 0, 2)
    )  # [p, c, d]

    idn = np.eye(N, dtype=np.float32)

    if "nc" not in _cache:
        _cache["nc"] = _build()
    nc = _cache["nc"]

    in_maps = [
        {
            "hid": hid_p[c],
            "ast": ast_p[c],
            "atr": atr_p[c],
            "emb": emb_p,
            "asc": asc,
            "idn": idn,
        }
        for c in range(NCORES)
    ]

    trace = os.environ.get("KERNEL_TRACE", "0") == "1"
    res = run_bass_kernel_spmd(nc, in_maps, core_ids=list(range(NCORES)), trace=trace)
    if trace:
        _cache["exec_time_ns"] = res.exec_time_ns
        _cache["trace"] = res.instructions_and_trace

    output = np.empty((B, N, D), np.float32)
    attr_sess = np.empty((B, N, D), np.float32)
    for c in range(NCORES):
        s = slice(c * B_LOC, (c + 1) * B_LOC)
        output[s] = res.results[c]["out"].transpose(1, 0, 2)  # [i,b,d] -> [b,i,d]
        attr_sess[s] = res.results[c]["att"].transpose(1, 2, 0)  # [d,b,n] -> [b,n,d]
    return output, attr_sess
